# revision 15
# baseline (speedup 1.0000x reference)
import sys

sys.path.insert(0, "/opt/trn_rl_repo")

import numpy as np

# Problem constants (hardcoded per contract)
B, L, C, K = 8, 16384, 64, 7
T = (L - 2 * K) // 2 + 1  # 8186
HALF = 4096               # t's per half (half-1 ragged: 8186-4096=4090, padded)
TC = 512                  # t-chunk
NCH = HALF // TC          # 8 chunks
WX = 4104                 # column width of folded x tensors (HALF + 8 pad)
LN_EPS = 1e-6

# xin (per-call x data, f16) column offsets
O_XE = 0
O_XO = WX                 # 4104
NX = 2 * WX               # 8208
# cons (device-resident weights, f16) column offsets
C_WT = 0
C_ID = C_WT + 64 * K      # 448
C_ON = C_ID + 128         # 576
C_CK = C_ON + 64          # 640
NCOL = C_CK + 64          # 704

_CACHE = {}


def _build(prelu_slope: float, need_lnsb: bool, need_cb: bool):
    import concourse.bacc as bacc
    import concourse.mybir as mybir
    import concourse.tile as tile

    f32 = mybir.dt.float32
    f16 = mybir.dt.float16
    AF = mybir.ActivationFunctionType
    OP = mybir.AluOpType

    nc = bacc.Bacc("TRN2", target_bir_lowering=False, debug=False, num_devices=8)

    # ---- DRAM parameters ----
    dXIN = nc.declare_dram_parameter("xin", [128, NX], f16, isOutput=False)
    dCONS = nc.declare_dram_parameter("cons", [128, NCOL], f16, isOutput=False)
    if need_lnsb or need_cb:
        dCST = nc.declare_dram_parameter("csts", [128, 4], f32, isOutput=False)
    dOUT = nc.declare_dram_parameter("out", [T, C], f16, isOutput=True)

    from contextlib import ExitStack

    with ExitStack() as es:
        tc = es.enter_context(tile.TileContext(nc))
        cp = es.enter_context(tc.tile_pool(name="const", bufs=1))
        gp = es.enter_context(tc.tile_pool(name="gps", bufs=2, space="PSUM"))
        yp = es.enter_context(tc.tile_pool(name="yps", bufs=1, space="PSUM"))
        zp = es.enter_context(tc.tile_pool(name="zps", bufs=1, space="PSUM"))
        sp = es.enter_context(tc.tile_pool(name="sps", bufs=1, space="PSUM"))
        hp = es.enter_context(tc.tile_pool(name="hsb", bufs=10))
        pp = es.enter_context(tc.tile_pool(name="prod", bufs=16))
        ypool = es.enter_context(tc.tile_pool(name="ysb", bufs=3))
        st1 = es.enter_context(tc.tile_pool(name="st1", bufs=3))
        st2 = es.enter_context(tc.tile_pool(name="st2", bufs=3))
        st3 = es.enter_context(tc.tile_pool(name="st3", bufs=3))
        st4 = es.enter_context(tc.tile_pool(name="st4", bufs=3))
        st5 = es.enter_context(tc.tile_pool(name="st5", bufs=3))
        ynp = es.enter_context(tc.tile_pool(name="ynp", bufs=3))
        pzp = es.enter_context(tc.tile_pool(name="pzp", bufs=3))
        trp = es.enter_context(tc.tile_pool(name="trp", bufs=6))
        op_ = es.enter_context(tc.tile_pool(name="outp", bufs=4))
        if True:
            # ---- load the input blob ----
            BL = cp.tile([128, NX], f16)
            nc.sync.dma_start(BL[:], dXIN[:])
            CO = cp.tile([128, NCOL], f16)
            nc.sync.dma_start(CO[:], dCONS[:])
            EPS = cp.tile([128, 1], f32)
            nc.gpsimd.memset(EPS[:], LN_EPS)
            if need_lnsb or need_cb:
                CST = cp.tile([128, 4], f32)
                nc.sync.dma_start(CST[:], dCST[:])

            for i in range(NCH):
                t0 = TC * i
                # ---- G matmuls + tanh: 7 m-planes, each (Ge|Go) (128,1024) ----
                hts = []
                for m in range(K):
                    g = gp.tile([128, 1024], f32)
                    for ci, xoff in ((0, O_XE), (512, O_XO)):
                        for h in (0, 1):
                            p0 = 64 * h
                            nc.tensor.matmul(
                                g[p0:p0 + 64, ci:ci + TC],
                                lhsT=CO[p0:p0 + 64, C_WT + 64 * m:C_WT + 64 * m + 64],
                                rhs=BL[p0:p0 + 64, xoff + t0 + 6:xoff + t0 + 6 + TC],
                                start=True, stop=True,
                            )
                    ht = hp.tile([128, 1024], f16)
                    nc.scalar.activation(ht[:], g[:], AF.Tanh)
                    hts.append(ht)

                # ---- gating products (14 planes) ----
                # shifted tensors eliminated: for all m the window operand is
                # x*[:, t0+m : t0+m+TC] (odd m read the +1-shifted column
                # range of the same folded tensor)
                prods = []
                for m in range(K):
                    for ci, xoff in ((0, O_XE), (512, O_XO)):
                        pr = pp.tile([128, TC], f16)
                        nc.vector.tensor_mul(
                            pr[:], BL[:, xoff + t0 + m:xoff + t0 + m + TC],
                            hts[m][:, ci:ci + TC])
                        prods.append(pr)

                # ---- accumulate 14 products + skip via identity matmuls ----
                y = yp.tile([128, TC], f32)
                for j, pr in enumerate(prods):
                    nc.tensor.matmul(y[:], lhsT=CO[:, C_ID:C_ID + 128], rhs=pr[:],
                                     start=(j == 0), stop=False)
                nc.tensor.matmul(y[:], lhsT=CO[:, C_ID:C_ID + 128],
                                 rhs=BL[:, O_XE + t0 + 6:O_XE + t0 + 6 + TC],
                                 start=False, stop=True)

                # ---- drain y, square ----
                ysb = ypool.tile([128, TC], f16)
                nc.scalar.copy(ysb[:], y[:])
                ysq = pp.tile([128, TC], f16)
                nc.vector.tensor_mul(ysq[:], ysb[:], ysb[:])

                # ---- LN stats: mean & mean-of-squares via ones-matmul ----
                st = sp.tile([128, 1024], f32)
                for h in (0, 1):
                    p0 = 64 * h
                    nc.tensor.matmul(st[p0:p0 + 64, 0:TC],
                                     lhsT=CO[p0:p0 + 64, C_ON:C_ON + 64],
                                     rhs=ysb[p0:p0 + 64, :], start=True, stop=True)
                    nc.tensor.matmul(st[p0:p0 + 64, 512:512 + TC],
                                     lhsT=CO[p0:p0 + 64, C_ON:C_ON + 64],
                                     rhs=ysq[p0:p0 + 64, :], start=True, stop=True)
                mu = st[:, 0:TC]
                m2 = st[:, 512:512 + TC]

                musq = st1.tile([128, TC], f32)
                nc.scalar.activation(musq[:], mu, AF.Square)
                var = st2.tile([128, TC], f32)
                nc.vector.tensor_sub(var[:], m2, musq[:])
                std = st3.tile([128, TC], f32)
                nc.scalar.activation(std[:], var[:], AF.Sqrt, bias=EPS[:, 0:1])
                rstd = st4.tile([128, TC], f32)
                scr = st5.tile([128, TC], f32)
                nc.vector.reciprocal_approx_accurate(rstd[:], std[:], scr[:])

                # ---- yn = (y - mu) * rstd  (* s + b) ----
                yc = st1.tile([128, TC], f32)
                nc.vector.tensor_sub(yc[:], ysb[:], mu)
                yn = ynp.tile([128, TC], f16)
                nc.vector.tensor_mul(yn[:], yc[:], rstd[:])
                if need_lnsb:
                    yn2 = ynp.tile([128, TC], f16)
                    nc.vector.tensor_scalar(yn2[:], yn[:], CST[:, 0:1], CST[:, 1:2],
                                            op0=OP.mult, op1=OP.add)
                    yn = yn2

                # ---- 1x1 conv ----
                z = zp.tile([128, TC], f32)
                for h in (0, 1):
                    p0 = 64 * h
                    nc.tensor.matmul(z[p0:p0 + 64, :],
                                     lhsT=CO[p0:p0 + 64, C_CK:C_CK + 64],
                                     rhs=yn[p0:p0 + 64, :], start=True, stop=True)
                if need_cb:
                    z2 = st2.tile([128, TC], f32)
                    nc.vector.tensor_scalar(z2[:], z[:], CST[:, 2:3], None, op0=OP.add)
                    zsrc = z2
                else:
                    zsrc = z
                # prelu: max(z, slope*z)
                pz = pzp.tile([128, TC], f16)
                nc.scalar.activation(pz[:], zsrc[:], AF.Prelu,
                                     alpha=float(prelu_slope))

                # ---- transpose yn, pz to t-layout; add; store ----
                for h in (0, 1):
                    p0 = 64 * h
                    tb = HALF * h + t0
                    ynT = trp.tile([128, 4, 64], f16)
                    nc.sync.dma_start_transpose(ynT[:], yn[p0:p0 + 64, :])
                    pzT = trp.tile([128, 4, 64], f16)
                    nc.sync.dma_start_transpose(pzT[:], pz[p0:p0 + 64, :])
                    of = op_.tile([128, 4, 64], f16)
                    nc.vector.tensor_add(of[:], ynT[:], pzT[:])
                    if tb + TC <= T:
                        dst = dOUT[tb:tb + TC, :].rearrange(
                            "(j p) c -> p j c", p=128)
                        nc.sync.dma_start(dst, of[:])
                    else:
                        nfull = (T - tb) // 128
                        rem = (T - tb) - nfull * 128
                        if nfull > 0:
                            dst = dOUT[tb:tb + nfull * 128, :].rearrange(
                                "(j p) c -> p j c", p=128)
                            nc.sync.dma_start(dst, of[:, 0:nfull, :])
                        if rem > 0:
                            dst = dOUT[tb + nfull * 128:T, :]
                            nc.sync.dma_start(dst, of[0:rem, nfull, :])

    nc.compile()
    return nc


def _make_runner(nc, n_cores=8):
    """Cached jit of the bass program: one sharded executable reused across
    calls, donated output buffers generated on-device (no H2D of zeros),
    outputs fetched per-shard concurrently."""
    import jax
    import jax.numpy as jnp
    from jax.sharding import Mesh, PartitionSpec, NamedSharding
    from jax.experimental.shard_map import shard_map
    import concourse.mybir as mybir
    from concourse.bass2jax import (
        install_neuronx_cc_hook, _bass_exec_p, partition_id_tensor)

    install_neuronx_cc_hook()
    assert nc.dbg_addr is None

    partition_name = nc.partition_id_tensor.name if nc.partition_id_tensor else None
    in_names, out_names, out_avals = [], [], []
    for alloc in nc.m.functions[0].allocations:
        if not isinstance(alloc, mybir.MemoryLocationSet):
            continue
        name = alloc.memorylocations[0].name
        if alloc.kind == "ExternalInput":
            if name != partition_name:
                in_names.append(name)
        elif alloc.kind == "ExternalOutput":
            out_avals.append(jax.core.ShapedArray(
                tuple(alloc.tensor_shape), mybir.dt.np(alloc.dtype)))
            out_names.append(name)
    n_params = len(in_names)
    n_outs = len(out_avals)
    in_names_all = list(in_names) + list(out_names)
    if partition_name is not None:
        in_names_all.append(partition_name)

    def _body(*args):
        operands = list(args)
        if partition_name is not None:
            operands.append(partition_id_tensor())
        return tuple(_bass_exec_p.bind(
            *operands,
            out_avals=tuple(out_avals),
            in_names=tuple(in_names_all),
            out_names=tuple(out_names),
            lowering_input_output_aliases=(),
            sim_require_finite=True,
            sim_require_nnan=True,
            nc=nc,
        ))

    devices = jax.devices()[:n_cores]
    mesh = Mesh(np.asarray(devices), ("core",))
    in_specs = (PartitionSpec("core"),) * (n_params + n_outs)
    out_specs = (PartitionSpec("core"),) * n_outs
    donate = tuple(range(n_params, n_params + n_outs))
    sharded = jax.jit(
        shard_map(_body, mesh=mesh, in_specs=in_specs, out_specs=out_specs,
                  check_rep=False),
        donate_argnums=donate, keep_unused=True)

    zsh = tuple(NamedSharding(mesh, PartitionSpec("core")) for _ in out_avals)
    mkzeros = jax.jit(
        lambda: tuple(jnp.zeros((n_cores * a.shape[0], *a.shape[1:]), a.dtype)
                      for a in out_avals),
        out_shardings=zsh)

    import os as _os
    dbg = bool(_os.environ.get("KERNEL_TIMING_DEBUG"))
    state = {"donate": None}

    def run(concat_inputs):
        """concat_inputs: list of np/device arrays, each
        (n_cores*per_core_rows, ...) in in_names order. Returns per-output
        list of per-core np arrays. The kernel writes every element of its
        outputs, so the previous call's (already-fetched) output buffers are
        recycled as the next call's donated output operands."""
        import time as _t
        t0 = _t.time()
        don = state["donate"]
        if don is None:
            don = mkzeros()
        outs = sharded(*concat_inputs, *don)
        state["donate"] = list(outs)
        t1 = _t.time()
        shard_lists = []
        for o in outs:
            shards = sorted(o.addressable_shards,
                            key=lambda s: s.index[0].start or 0)
            for s in shards:
                s.data.copy_to_host_async()
            shard_lists.append(shards)
        t2 = _t.time()
        res = [[np.asarray(s.data) for s in shards] for shards in shard_lists]
        if dbg:
            print(f"  [run] dispatch {t1-t0:.3f}s  async-start {t2-t1:.3f}s  "
                  f"fetch {_t.time()-t2:.3f}s")
        return res

    return run, list(in_names), mesh


def _prep_xin(x):
    """Host-side prep of per-call x data: one (8*128, NX) f16 array."""
    xf = np.asarray(x, dtype=np.float32)
    blob = np.zeros((B * 128, NX), np.float16)
    for bi in range(B):
        r = bi * 128
        xb = xf[bi]                                          # (L, C)
        xeT = np.ascontiguousarray(xb[0::2].T).astype(np.float16)  # (64, 8192)
        xoT = np.ascontiguousarray(xb[1::2].T).astype(np.float16)
        blob[r:r + 64, O_XE:O_XE + WX] = xeT[:, 0:WX]
        blob[r + 64:r + 128, O_XE:O_XE + 8192 - HALF] = xeT[:, HALF:8192]
        blob[r:r + 64, O_XO:O_XO + WX] = xoT[:, 0:WX]
        blob[r + 64:r + 128, O_XO:O_XO + 8192 - HALF] = xoT[:, HALF:8192]
    return blob


def _prep_cons(weights, conv_kernel):
    """(128, NCOL) f16 weights blob, identical for every core."""
    wt = np.zeros((128, 64 * K), np.float16)
    for m in range(K):
        wmT = np.asarray(weights[:, :, m]).T.astype(np.float16)  # (c_in, d)
        wt[0:64, 64 * m:64 * m + 64] = wmT
        wt[64:128, 64 * m:64 * m + 64] = wmT
    ident = np.eye(128, dtype=np.float16)
    ones = np.full((128, 64), 1.0 / 64, np.float16)
    ck = np.zeros((128, 64), np.float16)
    ckc = np.asarray(conv_kernel).astype(np.float16)  # (c, o), lhsT layout
    ck[0:64] = ckc
    ck[64:128] = ckc
    return np.concatenate([wt, ident, ones, ck], axis=1)  # (128, NCOL)


def _prep_csts(ln_scale, ln_bias, conv_bias):
    cst = np.zeros((128, 4), np.float32)
    s = np.asarray(ln_scale, np.float32)
    b = np.asarray(ln_bias, np.float32)
    cb = np.asarray(conv_bias, np.float32)
    cst[0:64, 0] = s
    cst[64:128, 0] = s
    cst[0:64, 1] = b
    cst[64:128, 1] = b
    cst[0:64, 2] = cb
    cst[64:128, 2] = cb
    cst[:, 3] = LN_EPS
    return np.concatenate([cst] * B, axis=0)  # (8*128, 4)


def _get(key):
    if key not in _CACHE:
        nc = _build(*key)
        run, in_names, mesh = _make_runner(nc)
        _CACHE[key] = (nc, run, in_names, mesh, {})
    return _CACHE[key]


def _device_cons(mesh, cons_cache, cons_np):
    """Keep the replicated weights blob device-resident across calls;
    re-upload only when the weight values change."""
    cached = cons_cache.get("cons")
    if cached is not None and np.array_equal(cached[0], cons_np):
        return cached[1]
    import jax
    from jax.sharding import NamedSharding, PartitionSpec
    sh = NamedSharding(mesh, PartitionSpec("core"))
    dev = jax.device_put(np.concatenate([cons_np] * B, axis=0), sh)
    jax.block_until_ready(dev)
    cons_cache["cons"] = (cons_np, dev)
    return dev


def kernel(x, weights, ln_scale, ln_bias, conv_kernel, conv_bias, prelu_slope):
    slope = float(np.asarray(prelu_slope))
    need_lnsb = not (np.allclose(np.asarray(ln_scale), 1.0)
                     and np.allclose(np.asarray(ln_bias), 0.0))
    need_cb = not np.allclose(np.asarray(conv_bias), 0.0)

    nc, run, in_names, mesh, cons_cache = _get((slope, need_lnsb, need_cb))
    ins = {"xin": _prep_xin(x),
           "cons": _device_cons(mesh, cons_cache, _prep_cons(weights, conv_kernel))}
    if need_lnsb or need_cb:
        ins["csts"] = _prep_csts(ln_scale, ln_bias, conv_bias)
    outs = run([ins[n] for n in in_names])
    return np.stack(outs[0], axis=0).astype(np.float32)


# revision 18
# speedup vs baseline: 1.0398x; 1.0398x over previous
import sys

sys.path.insert(0, "/opt/trn_rl_repo")

import numpy as np

# Problem constants (hardcoded per contract)
B, L, C, K = 8, 16384, 64, 7
T = (L - 2 * K) // 2 + 1  # 8186
HALF = 4096               # t's per half (half-1 ragged: 8186-4096=4090, padded)
TC = 512                  # t-chunk
NCH = HALF // TC          # 8 chunks
WX = 4104                 # column width of folded x tensors (HALF + 8 pad)
LN_EPS = 1e-6

# xin (per-call x data, f16) column offsets
O_XE = 0
O_XO = WX                 # 4104
NX = 2 * WX               # 8208
# cons (device-resident weights, f16) column offsets
C_WT = 0
C_ID = C_WT + 64 * K      # 448
C_ON = C_ID + 128         # 576
C_CK = C_ON + 64          # 640
NCOL = C_CK + 64          # 704

_CACHE = {}


def _build(prelu_slope: float, need_lnsb: bool, need_cb: bool, reps: int = 1):
    import concourse.bacc as bacc
    import concourse.mybir as mybir
    import concourse.tile as tile

    f32 = mybir.dt.float32
    f16 = mybir.dt.float16
    AF = mybir.ActivationFunctionType
    OP = mybir.AluOpType

    nc = bacc.Bacc("TRN2", target_bir_lowering=False, debug=False, num_devices=8)

    # ---- DRAM parameters ----
    dXIN = nc.declare_dram_parameter("xin", [128, NX], f16, isOutput=False)
    dCONS = nc.declare_dram_parameter("cons", [128, NCOL], f16, isOutput=False)
    if need_lnsb or need_cb:
        dCST = nc.declare_dram_parameter("csts", [128, 4], f32, isOutput=False)
    dOUT = nc.declare_dram_parameter("out", [T, C], f16, isOutput=True)

    from contextlib import ExitStack

    with ExitStack() as es:
        tc = es.enter_context(tile.TileContext(nc))
        cp = es.enter_context(tc.tile_pool(name="const", bufs=1))
        gp = es.enter_context(tc.tile_pool(name="gps", bufs=2, space="PSUM"))
        yp = es.enter_context(tc.tile_pool(name="yps", bufs=1, space="PSUM"))
        zp = es.enter_context(tc.tile_pool(name="zps", bufs=1, space="PSUM"))
        sp = es.enter_context(tc.tile_pool(name="sps", bufs=1, space="PSUM"))
        hp = es.enter_context(tc.tile_pool(name="hsb", bufs=10))
        pp = es.enter_context(tc.tile_pool(name="prod", bufs=16))
        ypool = es.enter_context(tc.tile_pool(name="ysb", bufs=3))
        st1 = es.enter_context(tc.tile_pool(name="st1", bufs=3))
        st2 = es.enter_context(tc.tile_pool(name="st2", bufs=3))
        st3 = es.enter_context(tc.tile_pool(name="st3", bufs=3))
        st4 = es.enter_context(tc.tile_pool(name="st4", bufs=3))
        st5 = es.enter_context(tc.tile_pool(name="st5", bufs=3))
        ynp = es.enter_context(tc.tile_pool(name="ynp", bufs=3))
        pzp = es.enter_context(tc.tile_pool(name="pzp", bufs=3))
        trp = es.enter_context(tc.tile_pool(name="trp", bufs=6))
        op_ = es.enter_context(tc.tile_pool(name="outp", bufs=4))
        if True:
            # ---- load the input blob ----
            BL = cp.tile([128, NX], f16)
            nc.sync.dma_start(BL[:], dXIN[:])
            CO = cp.tile([128, NCOL], f16)
            nc.sync.dma_start(CO[:], dCONS[:])
            EPS = cp.tile([128, 1], f32)
            nc.gpsimd.memset(EPS[:], LN_EPS)
            if need_lnsb or need_cb:
                CST = cp.tile([128, 4], f32)
                nc.sync.dma_start(CST[:], dCST[:])

            for i in range(NCH * reps):
                i, t0 = i % NCH, TC * (i % NCH)
                # ---- G matmuls + tanh: 7 m-planes, each (Ge|Go) (128,1024) ----
                hts = []
                for m in range(K):
                    g = gp.tile([128, 1024], f32)
                    for ci, xoff in ((0, O_XE), (512, O_XO)):
                        for h in (0, 1):
                            p0 = 64 * h
                            nc.tensor.matmul(
                                g[p0:p0 + 64, ci:ci + TC],
                                lhsT=CO[p0:p0 + 64, C_WT + 64 * m:C_WT + 64 * m + 64],
                                rhs=BL[p0:p0 + 64, xoff + t0 + 6:xoff + t0 + 6 + TC],
                                start=True, stop=True,
                            )
                    ht = hp.tile([128, 1024], f16)
                    nc.scalar.activation(ht[:], g[:], AF.Tanh)
                    hts.append(ht)

                # ---- gating products (14 planes) ----
                # shifted tensors eliminated: for all m the window operand is
                # x*[:, t0+m : t0+m+TC] (odd m read the +1-shifted column
                # range of the same folded tensor)
                prods = []
                for m in range(K):
                    for ci, xoff in ((0, O_XE), (512, O_XO)):
                        pr = pp.tile([128, TC], f16)
                        nc.vector.tensor_mul(
                            pr[:], BL[:, xoff + t0 + m:xoff + t0 + m + TC],
                            hts[m][:, ci:ci + TC])
                        prods.append(pr)

                # ---- accumulate 14 products + skip via identity matmuls ----
                y = yp.tile([128, TC], f32)
                for j, pr in enumerate(prods):
                    nc.tensor.matmul(y[:], lhsT=CO[:, C_ID:C_ID + 128], rhs=pr[:],
                                     start=(j == 0), stop=False)
                nc.tensor.matmul(y[:], lhsT=CO[:, C_ID:C_ID + 128],
                                 rhs=BL[:, O_XE + t0 + 6:O_XE + t0 + 6 + TC],
                                 start=False, stop=True)

                # ---- drain y, square ----
                ysb = ypool.tile([128, TC], f16)
                nc.scalar.copy(ysb[:], y[:])
                ysq = pp.tile([128, TC], f16)
                nc.vector.tensor_mul(ysq[:], ysb[:], ysb[:])

                # ---- LN stats: mean & mean-of-squares via ones-matmul ----
                st = sp.tile([128, 1024], f32)
                for h in (0, 1):
                    p0 = 64 * h
                    nc.tensor.matmul(st[p0:p0 + 64, 0:TC],
                                     lhsT=CO[p0:p0 + 64, C_ON:C_ON + 64],
                                     rhs=ysb[p0:p0 + 64, :], start=True, stop=True)
                    nc.tensor.matmul(st[p0:p0 + 64, 512:512 + TC],
                                     lhsT=CO[p0:p0 + 64, C_ON:C_ON + 64],
                                     rhs=ysq[p0:p0 + 64, :], start=True, stop=True)
                mu = st[:, 0:TC]
                m2 = st[:, 512:512 + TC]

                musq = st1.tile([128, TC], f32)
                nc.scalar.activation(musq[:], mu, AF.Square)
                var = st2.tile([128, TC], f32)
                nc.vector.tensor_sub(var[:], m2, musq[:])
                std = st3.tile([128, TC], f32)
                nc.scalar.activation(std[:], var[:], AF.Sqrt, bias=EPS[:, 0:1])
                rstd = st4.tile([128, TC], f32)
                scr = st5.tile([128, TC], f32)
                nc.vector.reciprocal_approx_accurate(rstd[:], std[:], scr[:])

                # ---- yn = (y - mu) * rstd  (* s + b) ----
                yc = st1.tile([128, TC], f32)
                nc.vector.tensor_sub(yc[:], ysb[:], mu)
                yn = ynp.tile([128, TC], f16)
                nc.vector.tensor_mul(yn[:], yc[:], rstd[:])
                if need_lnsb:
                    yn2 = ynp.tile([128, TC], f16)
                    nc.vector.tensor_scalar(yn2[:], yn[:], CST[:, 0:1], CST[:, 1:2],
                                            op0=OP.mult, op1=OP.add)
                    yn = yn2

                # ---- 1x1 conv ----
                z = zp.tile([128, TC], f32)
                for h in (0, 1):
                    p0 = 64 * h
                    nc.tensor.matmul(z[p0:p0 + 64, :],
                                     lhsT=CO[p0:p0 + 64, C_CK:C_CK + 64],
                                     rhs=yn[p0:p0 + 64, :], start=True, stop=True)
                if need_cb:
                    z2 = st2.tile([128, TC], f32)
                    nc.vector.tensor_scalar(z2[:], z[:], CST[:, 2:3], None, op0=OP.add)
                    zsrc = z2
                else:
                    zsrc = z
                # prelu: max(z, slope*z)
                pz = pzp.tile([128, TC], f16)
                nc.scalar.activation(pz[:], zsrc[:], AF.Prelu,
                                     alpha=float(prelu_slope))

                # ---- transpose yn, pz to t-layout; add; store ----
                for h in (0, 1):
                    p0 = 64 * h
                    tb = HALF * h + t0
                    ynT = trp.tile([128, 4, 64], f16)
                    nc.sync.dma_start_transpose(ynT[:], yn[p0:p0 + 64, :])
                    pzT = trp.tile([128, 4, 64], f16)
                    nc.sync.dma_start_transpose(pzT[:], pz[p0:p0 + 64, :])
                    of = op_.tile([128, 4, 64], f16)
                    nc.vector.tensor_add(of[:], ynT[:], pzT[:])
                    if tb + TC <= T:
                        dst = dOUT[tb:tb + TC, :].rearrange(
                            "(j p) c -> p j c", p=128)
                        nc.sync.dma_start(dst, of[:])
                    else:
                        nfull = (T - tb) // 128
                        rem = (T - tb) - nfull * 128
                        if nfull > 0:
                            dst = dOUT[tb:tb + nfull * 128, :].rearrange(
                                "(j p) c -> p j c", p=128)
                            nc.sync.dma_start(dst, of[:, 0:nfull, :])
                        if rem > 0:
                            dst = dOUT[tb + nfull * 128:T, :]
                            nc.sync.dma_start(dst, of[0:rem, nfull, :])

    nc.compile()
    return nc


def _make_runner(nc, n_cores=8):
    """Cached jit of the bass program: one sharded executable reused across
    calls, donated output buffers generated on-device (no H2D of zeros),
    outputs fetched per-shard concurrently."""
    import jax
    import jax.numpy as jnp
    from jax.sharding import Mesh, PartitionSpec, NamedSharding
    from jax.experimental.shard_map import shard_map
    import concourse.mybir as mybir
    from concourse.bass2jax import (
        install_neuronx_cc_hook, _bass_exec_p, partition_id_tensor)

    install_neuronx_cc_hook()
    assert nc.dbg_addr is None

    partition_name = nc.partition_id_tensor.name if nc.partition_id_tensor else None
    in_names, out_names, out_avals = [], [], []
    for alloc in nc.m.functions[0].allocations:
        if not isinstance(alloc, mybir.MemoryLocationSet):
            continue
        name = alloc.memorylocations[0].name
        if alloc.kind == "ExternalInput":
            if name != partition_name:
                in_names.append(name)
        elif alloc.kind == "ExternalOutput":
            out_avals.append(jax.core.ShapedArray(
                tuple(alloc.tensor_shape), mybir.dt.np(alloc.dtype)))
            out_names.append(name)
    n_params = len(in_names)
    n_outs = len(out_avals)
    in_names_all = list(in_names) + list(out_names)
    if partition_name is not None:
        in_names_all.append(partition_name)

    def _body(*args):
        operands = list(args)
        if partition_name is not None:
            operands.append(partition_id_tensor())
        return tuple(_bass_exec_p.bind(
            *operands,
            out_avals=tuple(out_avals),
            in_names=tuple(in_names_all),
            out_names=tuple(out_names),
            lowering_input_output_aliases=(),
            sim_require_finite=True,
            sim_require_nnan=True,
            nc=nc,
        ))

    devices = jax.devices()[:n_cores]
    mesh = Mesh(np.asarray(devices), ("core",))
    in_specs = (PartitionSpec("core"),) * (n_params + n_outs)
    out_specs = (PartitionSpec("core"),) * n_outs
    donate = tuple(range(n_params, n_params + n_outs))
    sharded = jax.jit(
        shard_map(_body, mesh=mesh, in_specs=in_specs, out_specs=out_specs,
                  check_rep=False),
        donate_argnums=donate, keep_unused=True)

    zsh = tuple(NamedSharding(mesh, PartitionSpec("core")) for _ in out_avals)
    mkzeros = jax.jit(
        lambda: tuple(jnp.zeros((n_cores * a.shape[0], *a.shape[1:]), a.dtype)
                      for a in out_avals),
        out_shardings=zsh)

    import os as _os
    dbg = bool(_os.environ.get("KERNEL_TIMING_DEBUG"))
    state = {"donate": None}

    def run(concat_inputs):
        """concat_inputs: list of np/device arrays, each
        (n_cores*per_core_rows, ...) in in_names order. Returns per-output
        list of per-core np arrays. The kernel writes every element of its
        outputs, so the previous call's (already-fetched) output buffers are
        recycled as the next call's donated output operands."""
        import time as _t
        t0 = _t.time()
        don = state["donate"]
        if don is None:
            don = mkzeros()
        outs = sharded(*concat_inputs, *don)
        state["donate"] = list(outs)
        t1 = _t.time()
        shard_lists = []
        for o in outs:
            shards = sorted(o.addressable_shards,
                            key=lambda s: s.index[0].start or 0)
            for s in shards:
                s.data.copy_to_host_async()
            shard_lists.append(shards)
        t2 = _t.time()
        res = [[np.asarray(s.data) for s in shards] for shards in shard_lists]
        if dbg:
            print(f"  [run] dispatch {t1-t0:.3f}s  async-start {t2-t1:.3f}s  "
                  f"fetch {_t.time()-t2:.3f}s")
        return res

    return run, list(in_names), mesh


def _prep_xin(x):
    """Host-side prep of per-call x data: one (8*128, NX) f16 array."""
    xf = np.asarray(x, dtype=np.float32)
    # (B, L, C) -> (B, 2, C, L//2): xr[b, 0] = x[b, 0::2].T, xr[b, 1] = x[b, 1::2].T
    xr = xf.reshape(B, L // 2, 2, C).transpose(0, 2, 3, 1).astype(np.float16)
    blob = np.zeros((B * 128, NX), np.float16)
    bv = blob.reshape(B, 2, 64, NX)
    for e, off in ((0, O_XE), (1, O_XO)):
        bv[:, 0, :, off:off + WX] = xr[:, e, :, 0:WX]
        bv[:, 1, :, off:off + 8192 - HALF] = xr[:, e, :, HALF:8192]
    return blob


def _prep_cons(weights, conv_kernel):
    """(128, NCOL) f16 weights blob, identical for every core."""
    wt = np.zeros((128, 64 * K), np.float16)
    for m in range(K):
        wmT = np.asarray(weights[:, :, m]).T.astype(np.float16)  # (c_in, d)
        wt[0:64, 64 * m:64 * m + 64] = wmT
        wt[64:128, 64 * m:64 * m + 64] = wmT
    ident = np.eye(128, dtype=np.float16)
    ones = np.full((128, 64), 1.0 / 64, np.float16)
    ck = np.zeros((128, 64), np.float16)
    ckc = np.asarray(conv_kernel).astype(np.float16)  # (c, o), lhsT layout
    ck[0:64] = ckc
    ck[64:128] = ckc
    return np.concatenate([wt, ident, ones, ck], axis=1)  # (128, NCOL)


def _prep_csts(ln_scale, ln_bias, conv_bias):
    cst = np.zeros((128, 4), np.float32)
    s = np.asarray(ln_scale, np.float32)
    b = np.asarray(ln_bias, np.float32)
    cb = np.asarray(conv_bias, np.float32)
    cst[0:64, 0] = s
    cst[64:128, 0] = s
    cst[0:64, 1] = b
    cst[64:128, 1] = b
    cst[0:64, 2] = cb
    cst[64:128, 2] = cb
    cst[:, 3] = LN_EPS
    return np.concatenate([cst] * B, axis=0)  # (8*128, 4)


def _get(key):
    if key not in _CACHE:
        nc = _build(*key)
        run, in_names, mesh = _make_runner(nc)
        _CACHE[key] = (nc, run, in_names, mesh, {})
    return _CACHE[key]


def _device_cons(mesh, cons_cache, cons_np):
    """Keep the replicated weights blob device-resident across calls;
    re-upload only when the weight values change."""
    cached = cons_cache.get("cons")
    if cached is not None and np.array_equal(cached[0], cons_np):
        return cached[1]
    import jax
    from jax.sharding import NamedSharding, PartitionSpec
    sh = NamedSharding(mesh, PartitionSpec("core"))
    dev = jax.device_put(np.concatenate([cons_np] * B, axis=0), sh)
    jax.block_until_ready(dev)
    cons_cache["cons"] = (cons_np, dev)
    return dev


def kernel(x, weights, ln_scale, ln_bias, conv_kernel, conv_bias, prelu_slope):
    slope = float(np.asarray(prelu_slope))
    need_lnsb = not (np.allclose(np.asarray(ln_scale), 1.0)
                     and np.allclose(np.asarray(ln_bias), 0.0))
    need_cb = not np.allclose(np.asarray(conv_bias), 0.0)

    nc, run, in_names, mesh, cons_cache = _get((slope, need_lnsb, need_cb))
    ins = {"xin": _prep_xin(x),
           "cons": _device_cons(mesh, cons_cache, _prep_cons(weights, conv_kernel))}
    if need_lnsb or need_cb:
        ins["csts"] = _prep_csts(ln_scale, ln_bias, conv_bias)
    outs = run([ins[n] for n in in_names])
    return np.stack(outs[0], axis=0).astype(np.float32)


# revision 19
# speedup vs baseline: 1.2574x; 1.2093x over previous
import sys

sys.path.insert(0, "/opt/trn_rl_repo")

import numpy as np

# Problem constants (hardcoded per contract)
B, L, C, K = 8, 16384, 64, 7
T = (L - 2 * K) // 2 + 1  # 8186
HALF = 4096               # t's per half (half-1 ragged: 8186-4096=4090, padded)
TC = 512                  # t-chunk
NCH = HALF // TC          # 8 chunks
WX = 4104                 # column width of folded x tensors (HALF + 8 pad)
LN_EPS = 1e-6

# xin (per-call x data, f16) column offsets
O_XE = 0
O_XO = WX                 # 4104
NX = 2 * WX               # 8208
# cons (device-resident weights, f16) column offsets
C_WT = 0
C_ID = C_WT + 64 * K      # 448
C_ON = C_ID + 128         # 576
C_CK = C_ON + 64          # 640
NCOL = C_CK + 64          # 704

_CACHE = {}


def _build(prelu_slope: float, need_lnsb: bool, need_cb: bool, reps: int = 1):
    import concourse.bacc as bacc
    import concourse.mybir as mybir
    import concourse.tile as tile

    f32 = mybir.dt.float32
    f16 = mybir.dt.float16
    AF = mybir.ActivationFunctionType
    OP = mybir.AluOpType

    nc = bacc.Bacc("TRN2", target_bir_lowering=False, debug=False, num_devices=8)

    # ---- DRAM parameters ----
    dXIN = nc.declare_dram_parameter("xin", [128, NX], f16, isOutput=False)
    dCONS = nc.declare_dram_parameter("cons", [128, NCOL], f16, isOutput=False)
    if need_lnsb or need_cb:
        dCST = nc.declare_dram_parameter("csts", [128, 4], f32, isOutput=False)
    dOUT = nc.declare_dram_parameter("out", [T, C], f16, isOutput=True)

    from contextlib import ExitStack

    with ExitStack() as es:
        tc = es.enter_context(tile.TileContext(nc))
        cp = es.enter_context(tc.tile_pool(name="const", bufs=1))
        gp = es.enter_context(tc.tile_pool(name="gps", bufs=2, space="PSUM"))
        yp = es.enter_context(tc.tile_pool(name="yps", bufs=1, space="PSUM"))
        zp = es.enter_context(tc.tile_pool(name="zps", bufs=1, space="PSUM"))
        sp = es.enter_context(tc.tile_pool(name="sps", bufs=1, space="PSUM"))
        hp = es.enter_context(tc.tile_pool(name="hsb", bufs=10))
        pp = es.enter_context(tc.tile_pool(name="prod", bufs=16))
        ypool = es.enter_context(tc.tile_pool(name="ysb", bufs=3))
        st1 = es.enter_context(tc.tile_pool(name="st1", bufs=3))
        st2 = es.enter_context(tc.tile_pool(name="st2", bufs=3))
        st3 = es.enter_context(tc.tile_pool(name="st3", bufs=3))
        st4 = es.enter_context(tc.tile_pool(name="st4", bufs=3))
        st5 = es.enter_context(tc.tile_pool(name="st5", bufs=3))
        ynp = es.enter_context(tc.tile_pool(name="ynp", bufs=3))
        pzp = es.enter_context(tc.tile_pool(name="pzp", bufs=3))
        trp = es.enter_context(tc.tile_pool(name="trp", bufs=6))
        op_ = es.enter_context(tc.tile_pool(name="outp", bufs=4))
        if True:
            # ---- load the input blob ----
            BL = cp.tile([128, NX], f16)
            nc.sync.dma_start(BL[:], dXIN[:])
            CO = cp.tile([128, NCOL], f16)
            nc.sync.dma_start(CO[:], dCONS[:])
            EPS = cp.tile([128, 1], f32)
            nc.gpsimd.memset(EPS[:], LN_EPS)
            if need_lnsb or need_cb:
                CST = cp.tile([128, 4], f32)
                nc.sync.dma_start(CST[:], dCST[:])

            for i in range(NCH * reps):
                i, t0 = i % NCH, TC * (i % NCH)
                # ---- G matmuls + tanh: 7 m-planes, each (Ge|Go) (128,1024) ----
                hts = []
                for m in range(K):
                    g = gp.tile([128, 1024], f32)
                    for ci, xoff in ((0, O_XE), (512, O_XO)):
                        for h in (0, 1):
                            p0 = 64 * h
                            nc.tensor.matmul(
                                g[p0:p0 + 64, ci:ci + TC],
                                lhsT=CO[p0:p0 + 64, C_WT + 64 * m:C_WT + 64 * m + 64],
                                rhs=BL[p0:p0 + 64, xoff + t0 + 6:xoff + t0 + 6 + TC],
                                start=True, stop=True,
                            )
                    ht = hp.tile([128, 1024], f16)
                    nc.scalar.activation(ht[:], g[:], AF.Tanh)
                    hts.append(ht)

                # ---- gating products (14 planes) ----
                # shifted tensors eliminated: for all m the window operand is
                # x*[:, t0+m : t0+m+TC] (odd m read the +1-shifted column
                # range of the same folded tensor)
                prods = []
                for m in range(K):
                    for ci, xoff in ((0, O_XE), (512, O_XO)):
                        pr = pp.tile([128, TC], f16)
                        nc.vector.tensor_mul(
                            pr[:], BL[:, xoff + t0 + m:xoff + t0 + m + TC],
                            hts[m][:, ci:ci + TC])
                        prods.append(pr)

                # ---- accumulate 14 products + skip via identity matmuls ----
                y = yp.tile([128, TC], f32)
                for j, pr in enumerate(prods):
                    nc.tensor.matmul(y[:], lhsT=CO[:, C_ID:C_ID + 128], rhs=pr[:],
                                     start=(j == 0), stop=False)
                nc.tensor.matmul(y[:], lhsT=CO[:, C_ID:C_ID + 128],
                                 rhs=BL[:, O_XE + t0 + 6:O_XE + t0 + 6 + TC],
                                 start=False, stop=True)

                # ---- drain y, square ----
                ysb = ypool.tile([128, TC], f16)
                nc.scalar.copy(ysb[:], y[:])
                ysq = pp.tile([128, TC], f16)
                nc.vector.tensor_mul(ysq[:], ysb[:], ysb[:])

                # ---- LN stats: mean & mean-of-squares via ones-matmul ----
                st = sp.tile([128, 1024], f32)
                for h in (0, 1):
                    p0 = 64 * h
                    nc.tensor.matmul(st[p0:p0 + 64, 0:TC],
                                     lhsT=CO[p0:p0 + 64, C_ON:C_ON + 64],
                                     rhs=ysb[p0:p0 + 64, :], start=True, stop=True)
                    nc.tensor.matmul(st[p0:p0 + 64, 512:512 + TC],
                                     lhsT=CO[p0:p0 + 64, C_ON:C_ON + 64],
                                     rhs=ysq[p0:p0 + 64, :], start=True, stop=True)
                mu = st[:, 0:TC]
                m2 = st[:, 512:512 + TC]

                musq = st1.tile([128, TC], f32)
                nc.scalar.activation(musq[:], mu, AF.Square)
                var = st2.tile([128, TC], f32)
                nc.vector.tensor_sub(var[:], m2, musq[:])
                std = st3.tile([128, TC], f32)
                nc.scalar.activation(std[:], var[:], AF.Sqrt, bias=EPS[:, 0:1])
                rstd = st4.tile([128, TC], f32)
                scr = st5.tile([128, TC], f32)
                nc.vector.reciprocal_approx_accurate(rstd[:], std[:], scr[:])

                # ---- yn = (y - mu) * rstd  (* s + b) ----
                yc = st1.tile([128, TC], f32)
                nc.vector.tensor_sub(yc[:], ysb[:], mu)
                yn = ynp.tile([128, TC], f16)
                nc.vector.tensor_mul(yn[:], yc[:], rstd[:])
                if need_lnsb:
                    yn2 = ynp.tile([128, TC], f16)
                    nc.vector.tensor_scalar(yn2[:], yn[:], CST[:, 0:1], CST[:, 1:2],
                                            op0=OP.mult, op1=OP.add)
                    yn = yn2

                # ---- 1x1 conv ----
                z = zp.tile([128, TC], f32)
                for h in (0, 1):
                    p0 = 64 * h
                    nc.tensor.matmul(z[p0:p0 + 64, :],
                                     lhsT=CO[p0:p0 + 64, C_CK:C_CK + 64],
                                     rhs=yn[p0:p0 + 64, :], start=True, stop=True)
                if need_cb:
                    z2 = st2.tile([128, TC], f32)
                    nc.vector.tensor_scalar(z2[:], z[:], CST[:, 2:3], None, op0=OP.add)
                    zsrc = z2
                else:
                    zsrc = z
                # prelu: max(z, slope*z)
                pz = pzp.tile([128, TC], f16)
                nc.scalar.activation(pz[:], zsrc[:], AF.Prelu,
                                     alpha=float(prelu_slope))

                # ---- transpose yn, pz to t-layout; add; store ----
                for h in (0, 1):
                    p0 = 64 * h
                    tb = HALF * h + t0
                    ynT = trp.tile([128, 4, 64], f16)
                    nc.sync.dma_start_transpose(ynT[:], yn[p0:p0 + 64, :])
                    pzT = trp.tile([128, 4, 64], f16)
                    nc.sync.dma_start_transpose(pzT[:], pz[p0:p0 + 64, :])
                    of = op_.tile([128, 4, 64], f16)
                    nc.vector.tensor_add(of[:], ynT[:], pzT[:])
                    if tb + TC <= T:
                        dst = dOUT[tb:tb + TC, :].rearrange(
                            "(j p) c -> p j c", p=128)
                        nc.sync.dma_start(dst, of[:])
                    else:
                        nfull = (T - tb) // 128
                        rem = (T - tb) - nfull * 128
                        if nfull > 0:
                            dst = dOUT[tb:tb + nfull * 128, :].rearrange(
                                "(j p) c -> p j c", p=128)
                            nc.sync.dma_start(dst, of[:, 0:nfull, :])
                        if rem > 0:
                            dst = dOUT[tb + nfull * 128:T, :]
                            nc.sync.dma_start(dst, of[0:rem, nfull, :])

    nc.compile()
    return nc


def _make_runner(nc, n_cores=8):
    """Cached jit of the bass program: one sharded executable reused across
    calls, donated output buffers generated on-device (no H2D of zeros),
    outputs fetched per-shard concurrently."""
    import jax
    import jax.numpy as jnp
    from jax.sharding import Mesh, PartitionSpec, NamedSharding
    from jax.experimental.shard_map import shard_map
    import concourse.mybir as mybir
    from concourse.bass2jax import (
        install_neuronx_cc_hook, _bass_exec_p, partition_id_tensor)

    install_neuronx_cc_hook()
    assert nc.dbg_addr is None

    partition_name = nc.partition_id_tensor.name if nc.partition_id_tensor else None
    in_names, out_names, out_avals = [], [], []
    for alloc in nc.m.functions[0].allocations:
        if not isinstance(alloc, mybir.MemoryLocationSet):
            continue
        name = alloc.memorylocations[0].name
        if alloc.kind == "ExternalInput":
            if name != partition_name:
                in_names.append(name)
        elif alloc.kind == "ExternalOutput":
            out_avals.append(jax.core.ShapedArray(
                tuple(alloc.tensor_shape), mybir.dt.np(alloc.dtype)))
            out_names.append(name)
    n_params = len(in_names)
    n_outs = len(out_avals)
    in_names_all = list(in_names) + list(out_names)
    if partition_name is not None:
        in_names_all.append(partition_name)

    def _body(*args):
        operands = list(args)
        if partition_name is not None:
            operands.append(partition_id_tensor())
        return tuple(_bass_exec_p.bind(
            *operands,
            out_avals=tuple(out_avals),
            in_names=tuple(in_names_all),
            out_names=tuple(out_names),
            lowering_input_output_aliases=(),
            sim_require_finite=True,
            sim_require_nnan=True,
            nc=nc,
        ))

    devices = jax.devices()[:n_cores]
    mesh = Mesh(np.asarray(devices), ("core",))
    in_specs = (PartitionSpec("core"),) * (n_params + n_outs)
    out_specs = (PartitionSpec("core"),) * n_outs
    donate = tuple(range(n_params, n_params + n_outs))
    sharded = jax.jit(
        shard_map(_body, mesh=mesh, in_specs=in_specs, out_specs=out_specs,
                  check_rep=False),
        donate_argnums=donate, keep_unused=True)

    zsh = tuple(NamedSharding(mesh, PartitionSpec("core")) for _ in out_avals)
    mkzeros = jax.jit(
        lambda: tuple(jnp.zeros((n_cores * a.shape[0], *a.shape[1:]), a.dtype)
                      for a in out_avals),
        out_shardings=zsh)

    import os as _os
    dbg = bool(_os.environ.get("KERNEL_TIMING_DEBUG"))
    state = {"donate": None}

    def run(concat_inputs):
        """concat_inputs: list of np/device arrays, each
        (n_cores*per_core_rows, ...) in in_names order. Returns per-output
        list of per-core np arrays. The kernel writes every element of its
        outputs, so the previous call's (already-fetched) output buffers are
        recycled as the next call's donated output operands."""
        import time as _t
        t0 = _t.time()
        don = state["donate"]
        if don is None:
            don = mkzeros()
        outs = sharded(*concat_inputs, *don)
        state["donate"] = list(outs)
        t1 = _t.time()
        shard_lists = []
        for o in outs:
            shards = sorted(o.addressable_shards,
                            key=lambda s: s.index[0].start or 0)
            for s in shards:
                s.data.copy_to_host_async()
            shard_lists.append(shards)
        t2 = _t.time()
        res = [[np.asarray(s.data) for s in shards] for shards in shard_lists]
        if dbg:
            print(f"  [run] dispatch {t1-t0:.3f}s  async-start {t2-t1:.3f}s  "
                  f"fetch {_t.time()-t2:.3f}s")
        return res

    return run, list(in_names), mesh


def _prep_xin(x):
    """Host-side prep of per-call x data: one (8*128, NX) f16 array."""
    xf = np.asarray(x, dtype=np.float32)
    # (B, L, C) -> (B, 2, C, L//2): xr[b, 0] = x[b, 0::2].T, xr[b, 1] = x[b, 1::2].T
    xr = xf.reshape(B, L // 2, 2, C).transpose(0, 2, 3, 1).astype(np.float16)
    blob = np.zeros((B * 128, NX), np.float16)
    bv = blob.reshape(B, 2, 64, NX)
    for e, off in ((0, O_XE), (1, O_XO)):
        bv[:, 0, :, off:off + WX] = xr[:, e, :, 0:WX]
        bv[:, 1, :, off:off + 8192 - HALF] = xr[:, e, :, HALF:8192]
    return blob


def _prep_cons(weights, conv_kernel):
    """(128, NCOL) f16 weights blob, identical for every core."""
    wt = np.zeros((128, 64 * K), np.float16)
    for m in range(K):
        wmT = np.asarray(weights[:, :, m]).T.astype(np.float16)  # (c_in, d)
        wt[0:64, 64 * m:64 * m + 64] = wmT
        wt[64:128, 64 * m:64 * m + 64] = wmT
    ident = np.eye(128, dtype=np.float16)
    ones = np.full((128, 64), 1.0 / 64, np.float16)
    ck = np.zeros((128, 64), np.float16)
    ckc = np.asarray(conv_kernel).astype(np.float16)  # (c, o), lhsT layout
    ck[0:64] = ckc
    ck[64:128] = ckc
    return np.concatenate([wt, ident, ones, ck], axis=1)  # (128, NCOL)


def _prep_csts(ln_scale, ln_bias, conv_bias):
    cst = np.zeros((128, 4), np.float32)
    s = np.asarray(ln_scale, np.float32)
    b = np.asarray(ln_bias, np.float32)
    cb = np.asarray(conv_bias, np.float32)
    cst[0:64, 0] = s
    cst[64:128, 0] = s
    cst[0:64, 1] = b
    cst[64:128, 1] = b
    cst[0:64, 2] = cb
    cst[64:128, 2] = cb
    cst[:, 3] = LN_EPS
    return np.concatenate([cst] * B, axis=0)  # (8*128, 4)


def _get(key):
    if key not in _CACHE:
        nc = _build(*key)
        run, in_names, mesh = _make_runner(nc)
        _CACHE[key] = (nc, run, in_names, mesh, {})
    return _CACHE[key]


def _device_cons(mesh, cons_cache, cons_np):
    """Keep the replicated weights blob device-resident across calls;
    re-upload only when the weight values change."""
    cached = cons_cache.get("cons")
    if cached is not None and np.array_equal(cached[0], cons_np):
        return cached[1]
    import jax
    from jax.sharding import NamedSharding, PartitionSpec
    sh = NamedSharding(mesh, PartitionSpec("core"))
    dev = jax.device_put(np.concatenate([cons_np] * B, axis=0), sh)
    jax.block_until_ready(dev)
    cons_cache["cons"] = (cons_np, dev)
    return dev


def kernel(x, weights, ln_scale, ln_bias, conv_kernel, conv_bias, prelu_slope):
    slope = float(np.asarray(prelu_slope))
    need_lnsb = not (np.allclose(np.asarray(ln_scale), 1.0)
                     and np.allclose(np.asarray(ln_bias), 0.0))
    need_cb = not np.allclose(np.asarray(conv_bias), 0.0)

    nc, run, in_names, mesh, cons_cache = _get((slope, need_lnsb, need_cb))
    ins = {"xin": _prep_xin(x),
           "cons": _device_cons(mesh, cons_cache, _prep_cons(weights, conv_kernel))}
    if need_lnsb or need_cb:
        ins["csts"] = _prep_csts(ln_scale, ln_bias, conv_bias)
    outs = run([ins[n] for n in in_names])
    res = np.empty((B, T, C), np.float32)
    for i, shard in enumerate(outs[0]):
        res[i] = shard  # f16 -> f32 cast directly into the output slot
    return res
